# revision 1
# baseline (speedup 1.0000x reference)
"""Trainium2 Bass/Tile kernel: two chained VALID 3x3 convolutions.

    x  [N,3,256,256] --conv(w1)--> h [N,64,254,254] --conv(w2)--> out [N,128,252,252]

Data-parallel over 8 NeuronCores: batch N=16 -> 2 images per core, conv
weights replicated.  Per core the convs are computed as implicit GEMMs on the
tensor engine:

  conv1: contraction over C0*3*3=27 goes on SBUF partitions (im2col buffer
         built with 9 strided DMAs), one matmul per 2-row output chunk.
  conv2: contraction over C1*9=576.  h is stored twice in SBUF: partitions
         0:64 hold h rows [y0, y0+TY+2), partitions 64:128 hold the same rows
         shifted down by one (row r := h row r+1).  A single K=128 matmul then
         computes a pair of taps (di,dj)+(di+1,dj) at once, so the 9 taps cost
         6 matmul passes (3 pairs + 3 K=64 singles) instead of 9.
         PSUM accumulates the 6 matmuls, DVE copies to SBUF, DMA to HBM.

MODE selects the matmul dtype:
  "bf16": inputs cast to bfloat16 host-side, 1 cycle/row on the PE,
          fp32 PSUM accumulation (measured scale-rel absmax err ~3.5e-3)
  "tf32": float32r, ~3 cycles/row measured (err ~3.6e-4)
"""

from contextlib import ExitStack

import ml_dtypes
import numpy as np

import concourse.bass as bass
import concourse.mybir as mybir
import concourse.tile as tile
import concourse.bass_utils as bass_utils
from concourse import bacc

N_CORES = 8
FULL_N = 16
C0, C1, C2 = 3, 64, 128

MODE = "bf16"


def _mm_dt():
    return mybir.dt.bfloat16 if MODE == "bf16" else mybir.dt.float32r


def _np_dt():
    return ml_dtypes.bfloat16 if MODE == "bf16" else np.float32


class Geom:
    def __init__(self, npc, h0, w0, ty):
        self.npc = npc          # images per core
        self.h0, self.w0 = h0, w0
        self.h1, self.w1 = h0 - 2, w0 - 2
        self.h2, self.w2 = h0 - 4, w0 - 4
        self.ty = ty            # conv2 output rows per strip
        assert ty % 2 == 0 and self.h2 % ty == 0


GEOM = Geom(npc=FULL_N // N_CORES, h0=256, w0=256, ty=28)


def _emit(ctx: ExitStack, tc: tile.TileContext, g: Geom, out, x, w1t, w2p, w2s,
          mm_dt):
    nc = tc.nc
    f32 = mybir.dt.float32
    TY, W1, W2 = g.ty, g.w1, g.w2

    wpool = ctx.enter_context(tc.tile_pool(name="weights", bufs=1))
    b1pool = ctx.enter_context(tc.tile_pool(name="b1", bufs=2))
    hpool = ctx.enter_context(tc.tile_pool(name="h", bufs=2))
    opool = ctx.enter_context(tc.tile_pool(name="o2", bufs=4))
    ps1 = ctx.enter_context(tc.tile_pool(name="ps1", bufs=2, space="PSUM"))
    ps2 = ctx.enter_context(tc.tile_pool(name="ps2", bufs=4, space="PSUM"))

    w1t_sb = wpool.tile([27, C1], mm_dt)
    nc.sync.dma_start(w1t_sb[:], w1t)
    w2p_sb = wpool.tile([128, 3, C2], mm_dt)
    nc.sync.dma_start(w2p_sb[:], w2p)
    w2s_sb = wpool.tile([C1, 3, C2], mm_dt)
    nc.sync.dma_start(w2s_sb[:], w2s)

    def conv1(n, y0):
        """Produce the doubled h strip for conv2 rows [y0, y0+TY)."""
        # im2col: partition (di*3+dj)*3+c holds x[c, y0+r+di, dj:dj+W1]
        B1 = b1pool.tile([27, TY + 2, W1], mm_dt, tag="b1")
        for t9 in range(9):
            di, dj = divmod(t9, 3)
            nc.sync.dma_start(
                B1[3 * t9:3 * t9 + 3],
                x[n, :, y0 + di:y0 + di + TY + 2, dj:dj + W1])
        H = hpool.tile([128, TY + 2, W1], mm_dt, tag="h")
        for r in range(0, TY + 2, 2):
            P1 = ps1.tile([C1, 2, W1], f32, tag="p1")
            nc.tensor.matmul(P1[:], w1t_sb[:], B1[:, r:r + 2, :],
                             start=True, stop=True)
            # casting copy into partitions 0:64 (h rows r, r+1)
            nc.vector.tensor_copy(H[0:C1, r:r + 2, :], P1[:])
            # row-shifted copy into partitions 64:128 (hB row r' = h row r'+1);
            # DMA cannot read PSUM, so source the freshly written hA rows
            if r == 0:
                nc.sync.dma_start(H[C1:128, 0:1, :], H[0:C1, 1:2, :])
            else:
                nc.sync.dma_start(H[C1:128, r - 1:r + 1, :], H[0:C1, r:r + 2, :])
        return H

    def conv2(n, y0, H):
        for t in range(0, TY, 2):
            P2 = ps2.tile([C2, 2, W2], f32, tag="p2")
            for dj in range(3):  # pairs: taps (0,dj) + (1,dj)
                nc.tensor.matmul(P2[:], w2p_sb[:, dj, :],
                                 H[:, t:t + 2, dj:dj + W2],
                                 start=(dj == 0), stop=False)
            for dj in range(3):  # singles: tap (2,dj)
                nc.tensor.matmul(P2[:], w2s_sb[:, dj, :],
                                 H[0:C1, t + 2:t + 4, dj:dj + W2],
                                 start=False, stop=(dj == 2))
            O2 = opool.tile([C2, 2, W2], f32, tag="o2")
            nc.vector.tensor_copy(O2[:], P2[:])
            nc.sync.dma_start(out[n, :, y0 + t:y0 + t + 2, :], O2[:])

    strips = [(n, y0) for n in range(g.npc) for y0 in range(0, g.h2, TY)]
    # software pipeline: emit conv1 of strip s+1 before conv2 of strip s so
    # the tensor engine never waits on the h copies of the strip it consumes
    Hcur = conv1(*strips[0])
    for i, (n, y0) in enumerate(strips):
        Hnext = conv1(*strips[i + 1]) if i + 1 < len(strips) else None
        conv2(n, y0, Hcur)
        Hcur = Hnext


def build(g: Geom = GEOM, mm_dt=None):
    if mm_dt is None:
        mm_dt = _mm_dt()
    nc = bacc.Bacc("TRN2", target_bir_lowering=False, debug=False,
                   num_devices=N_CORES)
    f32 = mybir.dt.float32
    x = nc.dram_tensor("x", [g.npc, C0, g.h0, g.w0], mm_dt,
                       kind="ExternalInput").ap()
    w1t = nc.dram_tensor("w1t", [27, C1], mm_dt, kind="ExternalInput").ap()
    w2p = nc.dram_tensor("w2p", [128, 3, C2], mm_dt, kind="ExternalInput").ap()
    w2s = nc.dram_tensor("w2s", [C1, 3, C2], mm_dt, kind="ExternalInput").ap()
    out = nc.dram_tensor("out", [g.npc, C2, g.h2, g.w2], f32,
                         kind="ExternalOutput").ap()
    with tile.TileContext(nc) as tc:
        with ExitStack() as ctx:
            _emit(ctx, tc, g, out, x, w1t, w2p, w2s, mm_dt)
    nc.compile()
    return nc


def host_round(a: np.ndarray) -> np.ndarray:
    """Cast fp32 to the matmul storage dtype (bf16 cast, or tf32 rounding)."""
    a = np.ascontiguousarray(a, dtype=np.float32)
    if MODE == "bf16":
        return a.astype(ml_dtypes.bfloat16)
    b = a.view(np.uint32).copy()
    b += 0xFFF + ((b >> 13) & 1)
    b &= np.uint32(0xFFFFE000)
    return b.view(np.float32)


def pack_weights(w1: np.ndarray, w2: np.ndarray):
    """Host-side repack so every device DMA is contiguous.

    w1t[p, o] = w1[o, c, di, dj] with p = (di*3+dj)*3 + c  (matches im2col)
    w2p[k, dj, o]: k<64 -> w2[o, k, 0, dj]; k>=64 -> w2[o, k-64, 1, dj]
    w2s[c, dj, o] = w2[o, c, 2, dj]
    """
    w1 = np.ascontiguousarray(np.asarray(w1), dtype=np.float32)
    w2 = np.ascontiguousarray(np.asarray(w2), dtype=np.float32)
    w1t = np.ascontiguousarray(w1.transpose(2, 3, 1, 0).reshape(27, C1))
    w2p = np.empty((128, 3, C2), np.float32)
    w2p[:C1] = w2[:, :, 0, :].transpose(1, 2, 0)
    w2p[C1:] = w2[:, :, 1, :].transpose(1, 2, 0)
    w2s = np.ascontiguousarray(w2[:, :, 2, :].transpose(1, 2, 0))
    return host_round(w1t), host_round(w2p), host_round(w2s)


_NC_CACHE: dict = {}


def _get_nc():
    key = ("main", MODE)
    if key not in _NC_CACHE:
        _NC_CACHE[key] = build()
    return _NC_CACHE[key]


def run(x, w1, w2, trace: bool = False):
    """Shard, run on 8 cores, gather.  Returns (out, BassKernelResults)."""
    x = np.ascontiguousarray(np.asarray(x), dtype=np.float32)
    assert x.shape == (FULL_N, C0, GEOM.h0, GEOM.w0), x.shape
    w1t, w2p, w2s = pack_weights(w1, w2)
    xs = host_round(x).reshape(N_CORES, GEOM.npc, C0, GEOM.h0, GEOM.w0)
    in_maps = [
        {"x": np.ascontiguousarray(xs[c]), "w1t": w1t, "w2p": w2p, "w2s": w2s}
        for c in range(N_CORES)
    ]
    nc = _get_nc()
    res = bass_utils.run_bass_kernel_spmd(
        nc, in_maps, core_ids=list(range(N_CORES)), trace=trace)
    out = np.concatenate([r["out"] for r in res.results], axis=0)
    return out, res


def kernel(x, w1, w2):
    out, _ = run(x, w1, w2, trace=False)
    return out



# revision 8
# speedup vs baseline: 1.7184x; 1.7184x over previous
"""Trainium2 Bass/Tile kernel: two chained VALID 3x3 convolutions.

    x  [N,3,256,256] --conv(w1)--> h [N,64,254,254] --conv(w2)--> out [N,128,252,252]

Data-parallel over 8 NeuronCores: batch N=16 -> 2 images per core, conv
weights replicated.  Per core the convs are computed as implicit GEMMs on the
tensor engine (PE observed pinned at the 1.2 GHz throttled clock in this
environment, ~420 ns per 504-column bf16 matmul, so the win is fewer/denser
passes, not HAM warmup).

  conv1: contraction over C0*3*3=27 on SBUF partitions (im2col buffer built
         with 9 strided DMAs).  Column-tiled pair of matmuls per 2-row chunk
         produces the *doubled* h layout directly in PSUM:
           partitions 0:64  <- h rows (r, r+1)     (tile_position (0,0))
           partitions 64:128<- h rows (r+1, r+2)   (tile_position (0,64))
         so no SBUF->SBUF row-shift DMA is needed.

  conv2: contraction over C1*9=576 = 4.5 x 128.  Per output row-pair tile:
         3 K=128 matmuls cover taps (0,dj)+(1,dj) using the doubled H.
         The leftover taps (2,dj) are K=64 singles; singles of TWO adjacent
         output tiles are row-group-packed (tile_position rows 0 vs 64) so
         they run concurrently in the PE array: 9 effective passes per two
         tiles instead of 12.  PSUM accumulates; DVE copies to SBUF; DMA out.

MODE "bf16": inputs cast to bfloat16 host-side, fp32 PSUM accumulation
(measured scale-rel absmax err ~3.5e-3).
"""

from contextlib import ExitStack

import ml_dtypes
import numpy as np

import concourse.bass as bass
import concourse.mybir as mybir
import concourse.tile as tile
import concourse.bass_utils as bass_utils
from concourse import bacc

N_CORES = 8
FULL_N = 16
C0, C1, C2 = 3, 64, 128

MODE = "bf16"


def _mm_dt():
    return mybir.dt.bfloat16 if MODE == "bf16" else mybir.dt.float32r


def _np_dt():
    return ml_dtypes.bfloat16 if MODE == "bf16" else np.float32


class Geom:
    def __init__(self, npc, h0, w0, ty):
        self.npc = npc          # images per core
        self.h0, self.w0 = h0, w0
        self.h1, self.w1 = h0 - 2, w0 - 2
        self.h2, self.w2 = h0 - 4, w0 - 4
        self.ty = ty            # conv2 output rows per strip
        assert ty % 4 == 0 and self.h2 % ty == 0


GEOM = Geom(npc=FULL_N // N_CORES, h0=256, w0=256, ty=28)


def _emit(ctx: ExitStack, tc: tile.TileContext, g: Geom, out, x, w1t, w2p, w2s2,
          mm_dt):
    nc = tc.nc
    f32 = mybir.dt.float32
    TY, W1, W2 = g.ty, g.w1, g.w2

    wpool = ctx.enter_context(tc.tile_pool(name="weights", bufs=1))
    b1pool = ctx.enter_context(tc.tile_pool(name="b1", bufs=3))
    hpool = ctx.enter_context(tc.tile_pool(name="h", bufs=2))
    opool = ctx.enter_context(tc.tile_pool(name="o2", bufs=4))
    ps1 = ctx.enter_context(tc.tile_pool(name="ps1", bufs=4, space="PSUM"))
    ps2 = ctx.enter_context(tc.tile_pool(name="ps2", bufs=4, space="PSUM"))

    w1t_sb = wpool.tile([27, C1], mm_dt)
    nc.sync.dma_start(w1t_sb[:], w1t)
    w2p_sb = wpool.tile([128, 3, C2], mm_dt)
    w2s_sb = wpool.tile([128, 3, C2], mm_dt)

    def load_b1(n, y0):
        """Issue the 9 im2col DMAs for strip (n, y0); returns the tile."""
        # partition (di*3+dj)*3+c holds x[c, y0+r+di, dj:dj+W1]
        B1 = b1pool.tile([27, TY + 2, W1], mm_dt, tag="b1")
        for t9 in range(9):
            di, dj = divmod(t9, 3)
            nc.sync.dma_start(
                B1[3 * t9:3 * t9 + 3],
                x[n, :, y0 + di:y0 + di + TY + 2, dj:dj + W1])
        return B1

    def alloc_h():
        return hpool.tile([128, TY + 2, W1], mm_dt, tag="h", name="H")

    def conv1_slot(B1, H, r):
        """Produce doubled h rows: top = (r, r+1), bottom = (r+1, r+2)."""
        last = r + 2 >= TY + 2  # bottom half would run past the strip
        P1 = ps1.tile([128, 2, W1], f32, tag="p1")
        # col group 0: h rows r, r+1 -> PSUM partitions 0:64
        nc.tensor.matmul(P1[0:C1], w1t_sb[:], B1[:, r:r + 2, :],
                         start=True, stop=True)
        if not last:
            # col group 64: h rows r+1, r+2 -> PSUM partitions 64:128
            nc.tensor.matmul(P1[C1:128], w1t_sb[:], B1[:, r + 1:r + 3, :],
                             start=True, stop=True)
            nc.scalar.copy(H[:, r:r + 2, :], P1[:])
        else:
            nc.scalar.copy(H[0:C1, r:r + 2, :], P1[0:C1])

    def conv2_pair(n, y0, H, tA):
        """Two output row-pair tiles (tA, tB=tA+2): 6 K=128 pair matmuls +
        6 K=64 singles packed two-at-a-time into PE row groups 0/64."""
        tB = tA + 2
        PA = ps2.tile([C2, 2, W2], f32, tag="p2")
        PB = ps2.tile([C2, 2, W2], f32, tag="p2")
        for dj in range(3):  # taps (0,dj)+(1,dj) for tile A
            nc.tensor.matmul(PA[:], w2p_sb[:, dj, :],
                             H[:, tA:tA + 2, dj:dj + W2],
                             start=(dj == 0), stop=False)
        for dj in range(3):  # taps (0,dj)+(1,dj) for tile B
            nc.tensor.matmul(PB[:], w2p_sb[:, dj, :],
                             H[:, tB:tB + 2, dj:dj + W2],
                             start=(dj == 0), stop=False)
        # singles: tap (2,dj).  Top half (rows r = h row r) serves tile A,
        # bottom half (rows r = h row r+1) serves tile B, concurrently.
        # Bottom rows only span 0..TY-1, so the last tile-pair flips:
        # B reads top (needs h row TY+1), A reads bottom.
        b_on_top = tB + 2 > TY - 1  # B's bottom rows tB+1..tB+2 out of range
        for dj in range(3):
            stop = dj == 2
            if not b_on_top:
                nc.tensor.matmul(PA[:], w2s_sb[0:C1, dj, :],
                                 H[0:C1, tA + 2:tA + 4, dj:dj + W2],
                                 start=False, stop=stop)
                nc.tensor.matmul(PB[:], w2s_sb[C1:128, dj, :],
                                 H[C1:128, tB + 1:tB + 3, dj:dj + W2],
                                 start=False, stop=stop)
            else:
                nc.tensor.matmul(PA[:], w2s_sb[C1:128, dj, :],
                                 H[C1:128, tA + 1:tA + 3, dj:dj + W2],
                                 start=False, stop=stop)
                nc.tensor.matmul(PB[:], w2s_sb[0:C1, dj, :],
                                 H[0:C1, tB + 2:tB + 4, dj:dj + W2],
                                 start=False, stop=stop)
        for t, P in ((tA, PA), (tB, PB)):
            O2 = opool.tile([C2, 2, W2], f32, tag="o2")
            nc.vector.tensor_copy(O2[:], P[:])
            # out-DMAs ride the (idle) gpsimd engine's queue so they don't
            # serialize behind the im2col loads on the sync queue
            nc.gpsimd.dma_start(out[n, :, y0 + t:y0 + t + 2, :], O2[:])

    strips = [(n, y0) for n in range(g.npc) for y0 in range(0, g.h2, TY)]
    n_c1 = (TY + 2 + 1) // 2          # conv1 slots per strip (r = 0,2..TY)
    # software pipeline, im2col prefetched a full strip early:
    #   strip s body: issue B1(s+2) DMAs, conv2(s) interleaved with conv1(s+1)
    # so B1(s+1) had all of strip s-1's compute (~17us) to stream in.
    B1 = {0: load_b1(*strips[0])}
    if len(strips) > 1:
        B1[1] = load_b1(*strips[1])
    # conv2 weights can land any time before conv2(0); queue them after B1(0,1)
    nc.sync.dma_start(w2p_sb[:], w2p)
    nc.sync.dma_start(w2s_sb[:], w2s2)
    Hcur = alloc_h()
    for r in range(0, TY + 2, 2):
        conv1_slot(B1[0], Hcur, r)
    for i, (n, y0) in enumerate(strips):
        nxt = strips[i + 1] if i + 1 < len(strips) else None
        if i + 2 < len(strips):
            B1[i + 2] = load_b1(*strips[i + 2])
        if nxt is not None:
            Hnxt = alloc_h()
        c1r = 0
        for tA in range(0, TY, 4):
            conv2_pair(n, y0, Hcur, tA)
            if nxt is not None:       # ~2 conv1 slots per tile-pair
                stop_r = min(n_c1, c1r + 2) if tA < TY - 4 else n_c1
                while c1r < stop_r:
                    conv1_slot(B1[i + 1], Hnxt, 2 * c1r)
                    c1r += 1
        B1.pop(i, None)
        if nxt is not None:
            Hcur = Hnxt


def build(g: Geom = GEOM, mm_dt=None):
    if mm_dt is None:
        mm_dt = _mm_dt()
    nc = bacc.Bacc("TRN2", target_bir_lowering=False, debug=False,
                   num_devices=N_CORES)
    f32 = mybir.dt.float32
    x = nc.dram_tensor("x", [g.npc, C0, g.h0, g.w0], mm_dt,
                       kind="ExternalInput").ap()
    w1t = nc.dram_tensor("w1t", [27, C1], mm_dt, kind="ExternalInput").ap()
    w2p = nc.dram_tensor("w2p", [128, 3, C2], mm_dt, kind="ExternalInput").ap()
    w2s2 = nc.dram_tensor("w2s2", [128, 3, C2], mm_dt,
                          kind="ExternalInput").ap()
    out = nc.dram_tensor("out", [g.npc, C2, g.h2, g.w2], f32,
                         kind="ExternalOutput").ap()
    with tile.TileContext(nc) as tc:
        with ExitStack() as ctx:
            _emit(ctx, tc, g, out, x, w1t, w2p, w2s2, mm_dt)
    nc.compile()
    return nc


def host_round(a: np.ndarray) -> np.ndarray:
    """Cast fp32 to the matmul storage dtype (bf16 cast, or tf32 rounding)."""
    a = np.ascontiguousarray(a, dtype=np.float32)
    if MODE == "bf16":
        return a.astype(ml_dtypes.bfloat16)
    b = a.view(np.uint32).copy()
    b += 0xFFF + ((b >> 13) & 1)
    b &= np.uint32(0xFFFFE000)
    return b.view(np.float32)


def pack_weights(w1: np.ndarray, w2: np.ndarray):
    """Host-side repack so every device DMA is contiguous.

    w1t[p, o] = w1[o, c, di, dj] with p = (di*3+dj)*3 + c  (matches im2col)
    w2p[k, dj, o]: k<64 -> w2[o, k, 0, dj]; k>=64 -> w2[o, k-64, 1, dj]
    w2s2[k, dj, o] = w2[o, k mod 64, 2, dj]  (tap-2 weights, both halves)
    """
    w1 = np.ascontiguousarray(np.asarray(w1), dtype=np.float32)
    w2 = np.ascontiguousarray(np.asarray(w2), dtype=np.float32)
    w1t = np.ascontiguousarray(w1.transpose(2, 3, 1, 0).reshape(27, C1))
    w2p = np.empty((128, 3, C2), np.float32)
    w2p[:C1] = w2[:, :, 0, :].transpose(1, 2, 0)
    w2p[C1:] = w2[:, :, 1, :].transpose(1, 2, 0)
    w2s = w2[:, :, 2, :].transpose(1, 2, 0)
    w2s2 = np.ascontiguousarray(np.concatenate([w2s, w2s], axis=0))
    return host_round(w1t), host_round(w2p), host_round(w2s2)


_NC_CACHE: dict = {}


def _get_nc():
    key = ("main", MODE)
    if key not in _NC_CACHE:
        _NC_CACHE[key] = build()
    return _NC_CACHE[key]


def run(x, w1, w2, trace: bool = False):
    """Shard, run on 8 cores, gather.  Returns (out, BassKernelResults)."""
    x = np.ascontiguousarray(np.asarray(x), dtype=np.float32)
    assert x.shape == (FULL_N, C0, GEOM.h0, GEOM.w0), x.shape
    w1t, w2p, w2s2 = pack_weights(w1, w2)
    xs = host_round(x).reshape(N_CORES, GEOM.npc, C0, GEOM.h0, GEOM.w0)
    in_maps = [
        {"x": np.ascontiguousarray(xs[c]), "w1t": w1t, "w2p": w2p,
         "w2s2": w2s2}
        for c in range(N_CORES)
    ]
    nc = _get_nc()
    res = bass_utils.run_bass_kernel_spmd(
        nc, in_maps, core_ids=list(range(N_CORES)), trace=trace)
    out = np.concatenate([r["out"] for r in res.results], axis=0)
    return out, res


def kernel(x, w1, w2):
    out, _ = run(x, w1, w2, trace=False)
    return out
